# revision 83
# baseline (speedup 1.0000x reference)
"""CrossAttention TRN2 kernel v3: 8-core SPMD, shard = (batch, S1-half).

Host pre-transposes q,k,v (-> qT/kT/vT chunk arrays) and pre-packs weights, so
the device does zero layout transposes. Per core:
  1. Load qT/kT/vT (f32r), packed Wq/Wk/Wv (f32r), Wo (bf16, [4,128,512]
     head-pair packed), identity (bf16) for PE transposes, ones row.
  2. Projections: qhT[h] [65,1024] / khT[h] [65,2048] (row 64 = negm / ones),
     vh[ti] [128, 8*65] bf16 (per-head 64 cols + ones col -> fused PV+rowsum).
  3. Per head: raw-S max pass ([s,t] psum, 2 DVE rowmax reduces + gpsimd
     min per (h,qi) unit). negm -> tiny DMA into qhT row 64.  S^T pass with
     K=65 (ones row in khT adds -max[s]), ACT exp(scale=1/8) -> P^T bf16.
     Head 0/1's units are pulled forward into the k-proj/v-proj slots with
     the negm-critical copies prioritized on DVE.
  4. PV in [s,p] layout: per s-chunk, matmul(o[s,65], lhsT=P^T[t,s-chunk],
     rhs=vh[t, h*65:...]) accumulating over 16 t-tiles; col 64 = softmax
     denominator (ones col of vh). K=128, M=128, N=65 -> half the PE cycles
     of the [p,s]-layout PV.
  5. Tail per head (spread into next head's loop): 2 raw ACT/DVE copies
     evacuate o psum to SBUF early, DVE reciprocal of the 8 denominator
     cols, 8 Pool tensor_scalar normalize-copies SBUF->SBUF bf16 into o_n2
     (head pairs share tiles); per pair (at odd heads 3/5/7): 8 PE
     transposes [128,128] + ACT copies -> oT_hn2[pair] [128, 1024].
  6. Final proj: out[s,d] = sum_pair oT_hn2[pair]^T @ Wo_pair: K=128 full,
     psum accumulate over 4 pairs -> 8 store DMAs. No transposes on host.
"""
import sys
import functools

sys.path.insert(0, "/opt/trn_rl_repo")
import numpy as np
from contextlib import ExitStack

B, S1, S2, D, H, P = 4, 2048, 2048, 512, 8, 64
SC = S1 // 2          # 1024 q rows per core
NCORES = 8
DCH = D // 128        # 4 d-chunks
QT = SC // 128        # 8 q s-tiles
TT = S2 // 128        # 16 t-tiles
NPAIR = H // 2


@functools.lru_cache(maxsize=1)
def _build():
    from concourse import bacc, tile, mybir

    f32 = mybir.dt.float32
    f32r = mybir.dt.float32r
    bf16 = mybir.dt.bfloat16

    nc = bacc.Bacc("TRN2", target_bir_lowering=False, debug=False)

    qT_d = nc.dram_tensor("qT", [DCH, 128, SC], f32r, kind="ExternalInput").ap()
    kT_d = nc.dram_tensor("kT", [DCH, 128, S2], f32r, kind="ExternalInput").ap()
    vT_d = nc.dram_tensor("vT", [DCH, 128, S2], bf16, kind="ExternalInput").ap()
    wq_d = nc.dram_tensor("wq", [DCH, 128, H * P], f32r, kind="ExternalInput").ap()
    wk_d = nc.dram_tensor("wk", [DCH, 128, H * P], f32r, kind="ExternalInput").ap()
    wv_d = nc.dram_tensor("wv", [DCH, 128, H * P], bf16, kind="ExternalInput").ap()
    wo_d = nc.dram_tensor("wo", [NPAIR, 128, D], bf16, kind="ExternalInput").ap()
    id_d = nc.dram_tensor("ident", [128, 128], bf16, kind="ExternalInput").ap()
    on_d = nc.dram_tensor("ones1", [1, S2], f32r, kind="ExternalInput").ap()
    out_d = nc.dram_tensor("out", [SC, D], f32, kind="ExternalOutput").ap()

    with tile.TileContext(nc) as tc, ExitStack() as ctx:
        # ---- persistent SBUF (allocate all tags up front) ----
        acts = ctx.enter_context(tc.tile_pool(name="acts", bufs=1))
        qhT = [acts.tile([65, SC], f32r, tag=f"qhT{h}", name=f"qhT{h}") for h in range(H)]
        khT = [acts.tile([65, S2], f32r, tag=f"khT{h}", name=f"khT{h}") for h in range(H)]
        vh = [acts.tile([128, H * 65], bf16, tag=f"vh{t}", name=f"vh{t}") for t in range(TT)]
        ident = acts.tile([128, 128], bf16, tag="ident", name="ident")
        oT_hn2 = [acts.tile([128, SC], bf16, tag=f"ohn{p}", name=f"ohn{p}")
                  for p in range(NPAIR)]

        small = ctx.enter_context(tc.tile_pool(name="small", bufs=6))

        # shared PSUM work pool: S^T tiles, maxpass tiles, transposes (3x2 banks)
        work_ps = ctx.enter_context(tc.tile_pool(name="work", bufs=3, space="PSUM"))



        def copy_split(dst0, src0, dst1, src1, urgent=False):
            """Two psum->sbuf copies.  Urgent ones (the hp==0 tiles feeding
            head 0's maxpass -> negm chain) go to DVE, which is idle early in
            setup; everything else queues on ACT."""
            if urgent:
                nc.vector.tensor_copy(dst0, src0)
                nc.vector.tensor_copy(dst1, src1)
            else:
                nc.scalar.copy(dst0, src0)
                nc.scalar.copy(dst1, src1)

        # ---- maxpass pieces (head h, interleaved into other loops) ----
        # Unit (h, qi) covers t 0..2048 in 2 halves (one piece per half).
        # Pool-mode (3/4 of units): each half's [128,1024] psum tile is folded
        # to [128,512] on Pool into mxc, one DVE reduce over [128,1024] SBUF.
        # DVE-mode (1/4): baseline path, 2 psum DVE reduces + gpsimd min.
        nacc = [small.tile([128, 1], f32, tag=f"nacc{q % 2}", name=f"nacc{q}") for q in range(QT)]

        def maxpass_piece(h, qi, half):
            mx = work_ps.tile([128, 1024], f32, tag="work", name=f"mx{h}_{qi}_{half}")
            for tb in range(2):
                nc.tensor.matmul(
                    mx[:, tb * 512:(tb + 1) * 512],
                    qhT[h][0:64, qi * 128:(qi + 1) * 128],
                    khT[h][0:64, half * 1024 + tb * 512: half * 1024 + (tb + 1) * 512],
                    start=True, stop=True,
                )
            negm_p = small.tile([128, 1], f32, tag="negp", name=f"negp{h}_{qi}_{half}")
            nc.vector.tensor_reduce(
                negm_p[:], mx[:], axis=mybir.AxisListType.X,
                op=mybir.AluOpType.max, negate=True,
            )
            if half == 0:
                nc.gpsimd.tensor_copy(nacc[qi][:], negm_p[:])
            else:
                nc.gpsimd.tensor_scalar_min(nacc[qi][:], negm_p[:], nacc[qi][:])
                negm_r = small.tile([128, 1], f32r, tag="negr", name=f"negr{h}_{qi}")
                nc.gpsimd.tensor_copy(negm_r[:], nacc[qi][:])
                nc.sync.dma_start(
                    qhT[h][64:65, qi * 128:(qi + 1) * 128], negm_r[:],
                )

        # ---- setup: loads + projections ----
        with tc.tile_pool(name="vph", bufs=1) as vpool, \
             tc.tile_pool(name="proj_ps", bufs=2, space="PSUM") as proj_ps:
            wv_sb = vpool.tile([128, DCH * H * P], bf16, tag="wv", name="wv_sb")
            vT_sb = vpool.tile([128, DCH * S2], bf16, tag="vT", name="vT_sb")
            with tc.tile_pool(name="kph", bufs=1) as kpool, \
                 tc.tile_pool(name="qph", bufs=1) as qpool:
                wq_sb = qpool.tile([128, DCH * H * P], f32r, tag="wq", name="wq_sb")
                qT_sb = qpool.tile([128, DCH * SC], f32r, tag="qT", name="qT_sb")
                wk_sb = kpool.tile([128, DCH * H * P], f32r, tag="wk", name="wk_sb")
                kT_sb = kpool.tile([128, DCH * S2], f32r, tag="kT", name="kT_sb")
                # q inputs first (interleaved so chunk 0 lands asap), then k,
                # then v and the small persistent loads — ALL bulk loads are
                # queued before any dependency-stalled negm DMA can head-block
                # the sync DMA queue.
                # Loads: q inputs first, then k by (tb, c) column blocks so
                # tb-major k-proj tiles stream as columns arrive, then v.
                for c in range(DCH):
                    nc.sync.dma_start(wq_sb[:, c * 512:(c + 1) * 512], wq_d[c])
                    nc.sync.dma_start(qT_sb[:, c * SC:(c + 1) * SC], qT_d[c])
                for c in range(DCH):
                    nc.sync.dma_start(wk_sb[:, c * 512:(c + 1) * 512], wk_d[c])
                for tb in range(4):
                    for c in range(DCH):
                        nc.sync.dma_start(
                            kT_sb[:, c * S2 + tb * 512:c * S2 + (tb + 1) * 512],
                            kT_d[c][:, tb * 512:(tb + 1) * 512])
                nc.sync.dma_start(ident[:], id_d)
                for h in range(H):
                    nc.sync.dma_start(khT[h][64:65, :], on_d)
                for c in range(DCH):
                    nc.sync.dma_start(wv_sb[:, c * 512:(c + 1) * 512], wv_d[c])
                for tb in range(4):
                    for c in range(DCH):
                        nc.sync.dma_start(
                            vT_sb[:, c * S2 + tb * 512:c * S2 + (tb + 1) * 512],
                            vT_d[c][:, tb * 512:(tb + 1) * 512])

                def q_tile(hp, sb, urgent=False):
                    ps = proj_ps.tile([128, 512], f32, tag="pp", name=f"qp{hp}_{sb}")
                    for c in range(DCH):
                        nc.tensor.matmul(
                            ps[:],
                            wq_sb[:, c * 512 + hp * 128: c * 512 + (hp + 1) * 128],
                            qT_sb[:, c * SC + sb * 512: c * SC + sb * 512 + 512],
                            start=(c == 0), stop=(c == DCH - 1),
                        )
                    copy_split(
                        qhT[2 * hp][0:64, sb * 512:(sb + 1) * 512], ps[0:64, :],
                        qhT[2 * hp + 1][0:64, sb * 512:(sb + 1) * 512], ps[64:128, :],
                        urgent=urgent,
                    )

                def k_tile(hp, tb, urgent=False):
                    ps = proj_ps.tile([128, 512], f32, tag="pp", name=f"kp{hp}_{tb}")
                    for c in range(DCH):
                        nc.tensor.matmul(
                            ps[:],
                            wk_sb[:, c * 512 + hp * 128: c * 512 + (hp + 1) * 128],
                            kT_sb[:, c * S2 + tb * 512: c * S2 + tb * 512 + 512],
                            start=(c == 0), stop=(c == DCH - 1),
                        )
                    copy_split(
                        khT[2 * hp][0:64, tb * 512:(tb + 1) * 512], ps[0:64, :],
                        khT[2 * hp + 1][0:64, tb * 512:(tb + 1) * 512], ps[64:128, :],
                        urgent=urgent,
                    )

                # Critical chain first: qhT[0/1] and khT[0/1] cols 0..1024
                # (urgent copies on then-idle DVE), then head 0's half-0
                # maxpass pieces interleaved with the non-urgent q tiles,
                # then khT[0/1]'s back half, then the rest of k-proj with the
                # half-0 stragglers.
                q_tile(0, 0, urgent=True)
                q_tile(0, 1, urgent=True)
                k_tile(0, 0, urgent=True)
                k_tile(0, 1, urgent=True)
                for j, (hp, sb) in enumerate([(1, 0), (1, 1), (2, 0), (2, 1),
                                              (3, 0), (3, 1)]):
                    q_tile(hp, sb)
                    if j >= 1:
                        maxpass_piece(0, j - 1, 0)
                k_tile(0, 2)
                k_tile(0, 3)
                for j, (hp, tb) in enumerate([(1, 0), (2, 0), (3, 0),
                                              (1, 1), (2, 1), (3, 1),
                                              (1, 2), (2, 2), (3, 2),
                                              (1, 3), (2, 3), (3, 3)]):
                    k_tile(hp, tb)
                    if j < 3:
                        maxpass_piece(0, 5 + j, 0)

            # v-proj (qpool/kpool closed) with the remaining maxpass units
            # interleaved: head 0 units 6..7 at ti 0..3, head 1 unit 0 at
            # ti 4..5.
            for ti in range(TT):
                ps = proj_ps.tile([128, 512], f32, tag="pp", name=f"vp{ti}")
                for c in range(DCH):
                    nc.tensor.matmul(
                        ps[:],
                        vT_sb[:, c * S2 + ti * 128: c * S2 + (ti + 1) * 128],
                        wv_sb[:, c * 512:(c + 1) * 512],
                        start=(c == 0), stop=(c == DCH - 1),
                    )
                vdst = vh[ti][:].rearrange("t (h q) -> t h q", h=H, q=65)
                nc.scalar.copy(vdst[:, :, 0:64],
                               ps[:].rearrange("t (h q) -> t h q", h=H, q=64))
                # ones col via exp(0*x)=1 reading ps: inherits the psum
                # tile's ordering, unlike a dependency-free memset (which
                # races with in-flight loads when the allocator aliases
                # this tile over a setup pool's space).
                nc.scalar.activation(vdst[:, :, 64:65].rearrange("t h q -> t (h q)"),
                                     ps[:, 0:8],
                                     mybir.ActivationFunctionType.Exp, scale=0.0)
                if ti < 8:
                    maxpass_piece(0, ti, 1)
                elif ti < 10:
                    maxpass_piece(1, 0, ti - 8)

        # ---- attention-phase pools (opened after setup pools freed) ----
        wo_pool = ctx.enter_context(tc.tile_pool(name="wop", bufs=1))
        wo_sb = [wo_pool.tile([128, D], bf16, tag=f"wo{p}", name=f"wo{p}")
                 for p in range(NPAIR)]
        for p in range(NPAIR):
            nc.sync.dma_start(wo_sb[p][:], wo_d[p])
        on2_pool = ctx.enter_context(tc.tile_pool(name="on2", bufs=2))
        rec_pool = ctx.enter_context(tc.tile_pool(name="rec", bufs=2))
        osb_pool = ctx.enter_context(tc.tile_pool(name="osb", bufs=2))
        pt_pool = ctx.enter_context(tc.tile_pool(name="pt", bufs=8))
        fin_pool = ctx.enter_context(tc.tile_pool(name="fin", bufs=3))

        # ---- attention ----
        o_tiles = {}
        on2_tiles = {}

        def ocol(sc):
            # chains 0-3 in bank pair 0 (cols 0..260), 4-7 in bank pair 1
            # (cols 512..772): no single matmul output crosses a 2KB bank line.
            return (sc // 4) * 512 + (sc % 4) * 65

        def pv(h, tj, ptile):
            # start=True zeroes the whole 2KB psum bank, so only the FIRST
            # chain of each bank (sc 0 and 4) may set it, at tj==0; the other
            # chains' first writes land on that bank-reset's pending-zero
            # bytes and read as zero.
            for sc in range(QT):
                nc.tensor.matmul(
                    o_tiles[h][:, ocol(sc):ocol(sc) + 65],
                    ptile[:, sc * 128:(sc + 1) * 128],
                    vh[tj][:, h * 65:(h + 1) * 65],
                    start=(tj == 0 and sc % 4 == 0), stop=(tj == TT - 1),
                    skip_group_check=True,
                )

        def emit_tail(h):
            """Tail of head h (spread into next head's loop): evacuate o psum
            raw to SBUF with 2 quick copies (frees the psum tile early), then
            reciprocal on DVE and 8 normalize-copies on the otherwise-idle
            Pool engine (SBUF-only there is legal)."""
            o_ps = o_tiles[h]
            o_sb = osb_pool.tile([128, 520], bf16, tag=f"osb{h % 2}", name=f"osb{h}")
            rec = rec_pool.tile([128, 8], f32, tag=f"rec{h % 2}", name=f"rec{h}")

            def p_copy_a():
                nc.scalar.copy(o_sb[:, 0:260], o_ps[:, 0:260])

            def p_copy_b():
                nc.vector.tensor_copy(o_sb[:, 260:520], o_ps[:, 512:772])

            def p_rec():
                nc.vector.reciprocal(
                    rec[:].rearrange("s (c q) -> s c q", c=QT, q=1),
                    o_sb[:].rearrange("s (c q) -> s c q", c=QT, q=65)[:, :, 64:65])
            cls = [p_copy_a, p_copy_b, p_rec]

            def mk_norm(sc):
                def p_norm():
                    on2 = on2_tiles[(h // 2, sc)]
                    nc.gpsimd.tensor_scalar_mul(
                        on2[:, (h % 2) * 64:(h % 2) * 64 + 64],
                        o_sb[:, sc * 65:sc * 65 + 64],
                        rec[:, sc:sc + 1],
                    )
                return p_norm
            cls.extend(mk_norm(sc) for sc in range(QT))
            return cls

        def emit_pair_tail(pr, host_head):
            """Transpose pair pr's normalized o_n2 tiles -> oT_hn2[pr].
            Transpose outputs borrow the HOST head's o psum tile's unused
            columns (832+, bank 1): safe because these closures run at slots
            >= 6, after that head's sc=4 PV chain start=True (slot 4) has
            already bank-reset bank 1, and the PV chains only touch cols
            < 772."""
            cls = []

            def mk_tp(sc):
                def p_tp():
                    on2 = on2_tiles[(pr, sc)]
                    tp = work_ps.tile([128, 256], f32, tag="work",
                                      name=f"tp{pr}_{sc}").bitcast(bf16)[:, 0:128]
                    nc.tensor.transpose(tp, on2[:], ident[:])
                    nc.scalar.copy(oT_hn2[pr][:, sc * 128:(sc + 1) * 128], tp)
                return p_tp
            cls.extend(mk_tp(sc) for sc in range(QT))
            return cls

        tail7 = []
        tail7_i = [0]
        with tc.tile_pool(name="o_ps", bufs=1, space="PSUM") as o_pool:
            for h in range(H):
                o_tiles[h] = o_pool.tile([128, 1024], f32, tag="o", name=f"o{h}")
                if h % 2 == 0:
                    for sc in range(QT):
                        on2_tiles[(h // 2, sc)] = on2_pool.tile(
                            [128, 128], bf16, tag=f"on2{sc}", name=f"on2_{h // 2}_{sc}")
                pts = {}
                tail_cl = emit_tail(h - 1) if h > 0 else []
                if h >= 3 and h % 2 == 1:
                    tail_cl = tail_cl + emit_pair_tail((h - 3) // 2, h)
                # o_ps is single-buffered: head h's PV must wait until head
                # h-1's normalize reads finish (~slot 5).  Emit PV batches
                # starting at slot START_PV (2 per slot to catch up) so the
                # in-order PE queue never blocks on the o tile.
                start_pv = 1 if h == 0 else 4
                pv_done = 0
                for ti in range(TT):
                    # maxpass piece first: its DVE reduce is the longest
                    # per-slot consumer, start it as early as possible.
                    # Pieces shifted 2 slots early: head h+1's qi=0 was
                    # emitted at ti 14/15 of head h-1, so the last negm DMA
                    # (qi=7) issues at ti=13 and its latency hides.
                    if h + 1 < H and ti <= 13:
                        maxpass_piece(h + 1, (ti + 2) // 2, (ti + 2) % 2)
                    if h + 2 < H and ti >= 14:
                        maxpass_piece(h + 2, 0, ti - 14)
                    # PV batches next: their ptiles are old, always ready.
                    if ti >= start_pv:
                        budget = (3 if h == H - 1 else 2) if h > 0 else 1
                        while budget > 0 and pv_done < ti:
                            pv(h, pv_done, pts[pv_done])
                            pv_done += 1
                            budget -= 1
                    # last head: start its own tail early (PE has slack in
                    # head 7's maxpass-free slots) so the final projection
                    # isn't serialized behind the loop.
                    if h == H - 1 and pv_done == TT and ti < TT - 1:
                        if not tail7:
                            tail7.extend(emit_tail(H - 1))
                            tail7.extend(emit_pair_tail(NPAIR - 1, H - 1))
                        for _ in range(3):
                            if tail7_i[0] < len(tail7):
                                tail7[tail7_i[0]]()
                                tail7_i[0] += 1
                    st = work_ps.tile([128, 1024], f32, tag="work", name=f"st{h}_{ti}")
                    for sb in range(2):
                        nc.tensor.matmul(
                            st[:, sb * 512:(sb + 1) * 512],
                            khT[h][0:65, ti * 128:(ti + 1) * 128],
                            qhT[h][0:65, sb * 512:(sb + 1) * 512],
                            start=True, stop=True,
                        )
                    ptile = pt_pool.tile([128, 1024], bf16, tag="pt", name=f"pt{h}_{ti}")
                    nc.scalar.activation(ptile[:], st[:], mybir.ActivationFunctionType.Exp,
                                         scale=0.125)
                    pts[ti] = ptile
                    if ti > 0:
                        # 2 closures/slot for the first 4 slots (evac copies,
                        # recip, norms), then 1/slot: spreads the pair-tail
                        # transposes' work-pool steals thinner (19 closures
                        # max over 15 slots)
                        idxs = (2 * (ti - 1), 2 * (ti - 1) + 1) if ti <= 4 \
                            else (ti + 3,)
                        for idx in idxs:
                            if idx < len(tail_cl):
                                tail_cl[idx]()
                while pv_done < TT:
                    pv(h, pv_done, pts[pv_done])
                    pv_done += 1
            # remainder of head 7's tail + pair 3 tail (most was emitted
            # inside head 7's loop; pairs 0-2 at heads 3/5/7)
            if not tail7:
                tail7.extend(emit_tail(H - 1))
                tail7.extend(emit_pair_tail(NPAIR - 1, H - 1))
            while tail7_i[0] < len(tail7):
                tail7[tail7_i[0]]()
                tail7_i[0] += 1

        # ---- final projection: out[s,d] = sum_pair oT_hn2[p]^T @ Wo_p ----
        with tc.tile_pool(name="fin_ps", bufs=2, space="PSUM") as fin_ps:
            for sc in range(QT):
                fp = fin_ps.tile([128, 512], f32, tag="fp", name=f"fp{sc}")
                for pr in range(NPAIR):
                    nc.tensor.matmul(
                        fp[:],
                        oT_hn2[pr][:, sc * 128:(sc + 1) * 128],
                        wo_sb[pr][:],
                        start=(pr == 0), stop=(pr == NPAIR - 1),
                    )
                fin = fin_pool.tile([128, 512], f32, tag="fin", name=f"fin{sc}")
                if sc % 2 == 0:
                    nc.vector.tensor_copy(fin[:], fp[:])
                else:
                    nc.scalar.copy(fin[:], fp[:])
                nc.sync.dma_start(out_d[sc * 128:(sc + 1) * 128, :], fin[:])

    nc.compile()
    return nc


def _host_prep(q, k, v, Wq, Wk, Wv, Wo):
    import ml_dtypes
    wq_a = np.ascontiguousarray(
        Wq.transpose(1, 0, 2).reshape(DCH, 128, H * P), dtype=np.float32)
    wk_a = np.ascontiguousarray(
        Wk.transpose(1, 0, 2).reshape(DCH, 128, H * P), dtype=np.float32)
    wv_a = np.ascontiguousarray(
        Wv.transpose(1, 0, 2).reshape(DCH, 128, H * P)).astype(ml_dtypes.bfloat16)
    wo_a = np.ascontiguousarray(Wo.reshape(NPAIR, 128, D)).astype(ml_dtypes.bfloat16)
    ident = np.eye(128, dtype=ml_dtypes.bfloat16)
    ones1 = np.ones((1, S2), dtype=np.float32)
    in_maps = []
    for c in range(NCORES):
        b, half = c // 2, c % 2
        qT = np.ascontiguousarray(
            q[b, half * SC:(half + 1) * SC, :].T.reshape(DCH, 128, SC))
        kT = np.ascontiguousarray(k[b].T.reshape(DCH, 128, S2))
        vT = np.ascontiguousarray(
            v[b].T.reshape(DCH, 128, S2)).astype(ml_dtypes.bfloat16)
        in_maps.append({
            "qT": qT, "kT": kT, "vT": vT,
            "wq": wq_a, "wk": wk_a, "wv": wv_a, "wo": wo_a,
            "ident": ident, "ones1": ones1,
        })
    return in_maps


def kernel(q, k, v, Wq, Wk, Wv, Wo):
    nc = _build()
    from concourse.bass_utils import run_bass_kernel_spmd

    q = np.asarray(q, np.float32)
    k = np.asarray(k, np.float32)
    v = np.asarray(v, np.float32)
    in_maps = _host_prep(q, k, v, np.asarray(Wq, np.float32),
                         np.asarray(Wk, np.float32), np.asarray(Wv, np.float32),
                         np.asarray(Wo, np.float32))
    res = run_bass_kernel_spmd(nc, in_maps, core_ids=list(range(NCORES)))
    globals()["LAST_RES"] = res
    out = np.empty((B, S1, D), np.float32)
    for c, r in enumerate(res.results):
        b, half = c // 2, c % 2
        out[b, half * SC:(half + 1) * SC] = r["out"]
    return out


if __name__ == "__main__":
    rng = np.random.default_rng(0)
    qq = rng.standard_normal((B, S1, D), dtype=np.float32)
    kk = rng.standard_normal((B, S2, D), dtype=np.float32)
    vv = rng.standard_normal((B, S2, D), dtype=np.float32)
    wq = rng.standard_normal((H, D, P), dtype=np.float32)
    wk = rng.standard_normal((H, D, P), dtype=np.float32)
    wv = rng.standard_normal((H, D, P), dtype=np.float32)
    wo = rng.standard_normal((H * P, D), dtype=np.float32)
    o = kernel(qq, kk, vv, wq, wk, wv, wo)
    print("out", o.shape, o.dtype, np.abs(o).mean())
